# revision 21
# baseline (speedup 1.0000x reference)
"""Trainium2 Bass kernel for nn_DistanceNetwork (retrieval_knn).

out[b, s, j] = dot[s, j] / (||sup[s, b]|| * ||inp[b]|| + EPS)
  dot[s, j] = sum_d sup[s, j, d] * inp[j, d]

Sharding: S=8192 split across 8 cores (1024 each). Each core reads its
support slice + the full input_signal, writes its [B, 1024, B] output
slice; host concatenates along axis 1.

Engine split per 128-s tile (layout [128 part = s, free = (b d)]):
 - DVE: fused mul+cumsum custom op (DOT_SCAN) -> per-segment dot via
   strided cumsum differences; SQ_SCAN cumsum of squares for the first
   K_DVE b-segments; small fixup ops.
 - ACT: Square+accumulate for the remaining b-segments' norms; sqrt.
 - GpSimd: the [B,B] outer-product broadcast multiply.
 - HWDGE (sync) DMAs.
"""

import os
import sys

import numpy as np

for _p in ("/opt/trn_rl_repo", "/root/.axon_site/_ro/trn_rl_repo"):
    if os.path.isdir(_p) and _p not in sys.path:
        sys.path.insert(0, _p)

import concourse.bass as bass
import concourse.bacc as bacc
import concourse.mybir as mybir
from concourse.bass_utils import run_bass_kernel_spmd, dve_ver_for
from concourse.tile import TileContext

S, B, D = 8192, 32, 128
NCORES = 8
SL = S // NCORES          # 1024 s-rows per core
P = 128                   # partition tile of s
TILES = SL // P           # 8 s-tiles per core
BD = B * D                # 4096
EPS = 1e-10
F32 = mybir.dt.float32
X = mybir.AxisListType.X

# How many of the 32 b-segments' sum-of-squares DVE computes (via SQ_SCAN);
# the rest go to the Scalar engine as Square+accumulate chunks.
K_DVE = 19
KD = K_DVE * D


# --- custom DVE ops (registered at import; uop table is built per-NEFF) --- #

def _register_scan_ops():
    import concourse.dve_ops as dve_ops_mod
    from concourse.dve_ops import DveOp, OPS, CUSTOM_DVE_SPECS
    from concourse.dve_spec import Spec, Src0, Src1, AluOp, scan, sq, lower
    from concourse.dve_spec import _has_src1
    from concourse.dve_uop import DveOpSpec

    def reg(name, spec):
        if name in dve_ops_mod._SUB_OPCODE_FOR_NAME:
            return next(op for op in OPS if op.name == name)
        op = DveOp(name=name, spec=spec, subdim=False, uops_sha={})
        OPS.append(op)
        CUSTOM_DVE_SPECS[name] = spec
        row = dve_ops_mod._CUSTOM_DVE_ROW_BASE + len(OPS) - 1
        assert row < 0x20
        dve_ops_mod._SUB_OPCODE_FOR_NAME[name] = row
        for ver in ("v3", "v4"):
            try:
                spec_c = DveOpSpec(
                    name=name,
                    opcode=row,
                    uops=lower(spec, ver=ver),
                    rd1_en=_has_src1(spec),
                )
                op.uops_sha[ver] = spec_c.sha(ver)
            except Exception:
                pass
        return op

    dot_scan = reg(
        "ANTK_DOT_SCAN",
        Spec(
            body=scan(AluOp.ADD, Src0 * Src1),
            reference=lambda in0, in1, s0, s1, imm2: np.cumsum(
                in0.astype(np.float32) * in1.astype(np.float32), axis=-1
            ),
        ),
    )
    sq_scan = reg(
        "ANTK_SQ_SCAN",
        Spec(
            body=scan(AluOp.ADD, sq(Src0)),
            reference=lambda in0, in1, s0, s1, imm2: np.cumsum(
                np.square(in0.astype(np.float32)), axis=-1
            ),
        ),
    )
    return dot_scan, sq_scan


DOT_SCAN, SQ_SCAN = _register_scan_ops()


def _build_nc():
    nc = bacc.Bacc()
    sup = nc.declare_dram_parameter("support", [SL, B, D], F32, isOutput=False)
    inp = nc.declare_dram_parameter("inp", [B, D], F32, isOutput=False)
    out = nc.declare_dram_parameter("out", [B, SL, B], F32, isOutput=True)
    SQUARE = mybir.ActivationFunctionType.Square

    with TileContext(nc) as tc:
        with (
            tc.tile_pool(name="const", bufs=1) as cpool,
            tc.tile_pool(name="sup", bufs=TILES) as suppool,
            tc.tile_pool(name="scan", bufs=2) as scpool,
            tc.tile_pool(name="outp", bufs=2) as opool,
            tc.tile_pool(name="small", bufs=3) as spool,
            tc.tile_pool(name="psum", bufs=1, space="PSUM") as ppool,
        ):
            # input_signal broadcast to all 128 partitions: [128, (b d)].
            # Read the 16 KiB once from HBM, then replicate across partitions
            # with K=1 ones-matmuls into PSUM (PE is otherwise idle; saves
            # both HBM broadcast traffic and 16 KiB/partition of SBUF).
            ones_l = cpool.tile([1, P], F32)
            nc.gpsimd.memset(ones_l[:], 1.0)
            inp_rep = ppool.tile([P, BD], F32)
            NBANK = 512
            # dummy matmul: eats the PE cold-start before inp_one arrives
            nc.tensor.matmul(
                inp_rep[0:1, 0:1], ones_l[:, 0:1], ones_l[:, 0:1],
                start=True, stop=True,
            )
            inp_one = scpool.tile([1, BD], F32, tag="dscan")
            with tc.high_priority():
                nc.sync.dma_start(
                    out=inp_one[:],
                    in_=inp[:, :].rearrange("b d -> (b d)").unsqueeze(0),
                )
                for k in range(BD // NBANK):
                    nc.tensor.matmul(
                        inp_rep[:, k * NBANK:(k + 1) * NBANK],
                        ones_l[:],
                        inp_one[:, k * NBANK:(k + 1) * NBANK],
                        start=True,
                        stop=True,
                    )
            tnorm = cpool.tile([P, B], F32)

            for t in range(TILES):
                sup_t = suppool.tile([P, BD], F32, tag="sup")
                nc.sync.dma_start(
                    out=sup_t[:],
                    in_=sup[t * P:(t + 1) * P, :, :].rearrange("s b d -> s (b d)"),
                )

                # sq[p, b]: first K_DVE segments on DVE (cumsum of squares),
                # the rest on ACT (Square with accumulate), 128 elems each.
                sq = spool.tile([P, B], F32, tag="sq")
                ssc = scpool.tile([P, KD + 1], F32, tag="sscan")
                nc.gpsimd.memset(ssc[:, 0:1], 0.0)
                nc.vector._custom_dve(
                    SQ_SCAN, out=ssc[:, 1:KD + 1], in0=sup_t[:, 0:KD]
                )
                sends = ssc[:, 1:KD + 1].rearrange("p (b d) -> p b d", d=D)
                sprevs = ssc[:, 0:KD].rearrange("p (b d) -> p b d", d=D)
                nc.gpsimd.tensor_sub(
                    sq[:, 0:K_DVE],
                    sends[:, :, D - 1:D].squeeze(2),
                    sprevs[:, :, 0:1].squeeze(2),
                )
                scr = spool.tile([P, D], F32, tag="scr")
                for b in range(K_DVE, B):
                    nc.scalar.activation(
                        scr[:],
                        sup_t[:, b * D:(b + 1) * D],
                        SQUARE,
                        accum_out=sq[:, b:b + 1],
                    )

                # dot[p, j]: cumsum of sup*inp along (b d); per-segment sums
                # are differences of the padded cumsum at segment boundaries.
                dsc = scpool.tile([P, BD + 1], F32, tag="dscan")
                nc.gpsimd.memset(dsc[:, 0:1], 0.0)
                nc.vector._custom_dve(
                    DOT_SCAN, out=dsc[:, 1:BD + 1], in0=sup_t[:], in1=inp_rep[:]
                )
                dot = spool.tile([P, B], F32, tag="dot")
                ends = dsc[:, 1:BD + 1].rearrange("p (b d) -> p b d", d=D)
                prevs = dsc[:, 0:BD].rearrange("p (b d) -> p b d", d=D)
                nc.gpsimd.tensor_sub(
                    dot[:], ends[:, :, D - 1:D].squeeze(2), prevs[:, :, 0:1].squeeze(2)
                )


                if t == 0:
                    # tnorm[p, b] = ||inp[b]|| — emitted after tile-0's main
                    # ops so the scheduler doesn't stall tile 0 behind it
                    itmp = scpool.tile([P, BD + 1], F32, tag="dscan")
                    nc.gpsimd.memset(itmp[:, 0:1], 0.0)
                    nc.vector._custom_dve(
                        SQ_SCAN, out=itmp[:, 1:BD + 1], in0=inp_rep[:]
                    )
                    tn2 = cpool.tile([P, B], F32)
                    iends = itmp[:, 1:BD + 1].rearrange("p (b d) -> p b d", d=D)
                    iprevs = itmp[:, 0:BD].rearrange("p (b d) -> p b d", d=D)
                    nc.gpsimd.tensor_sub(
                        tn2[:],
                        iends[:, :, D - 1:D].squeeze(2),
                        iprevs[:, :, 0:1].squeeze(2),
                    )
                    nc.scalar.sqrt(tnorm[:], tn2[:])

                # rden = 1 / ((sqrt(sq) + EPS') * tnorm)  (EPS folded in)
                sn = spool.tile([P, B], F32, tag="sn")
                nc.scalar.sqrt(sn[:], sq[:])
                den = spool.tile([P, B], F32, tag="den")
                nc.vector.scalar_tensor_tensor(
                    out=den[:],
                    in0=sn[:],
                    scalar=EPS,
                    in1=tnorm[:],
                    op0=mybir.AluOpType.add,
                    op1=mybir.AluOpType.mult,
                )
                rden = spool.tile([P, B], F32, tag="rden")
                nc.vector.reciprocal(rden[:], den[:])

                # outt[p, b, j] = rden[p, b] * dot[p, j]   (GpSimd)
                outt = opool.tile([P, B * B], F32, tag="outt")
                nc.gpsimd.tensor_mul(
                    outt[:].rearrange("p (b j) -> p b j", j=B),
                    rden[:].unsqueeze(2).broadcast_to([P, B, B]),
                    dot[:].unsqueeze(1).broadcast_to([P, B, B]),
                )
                # SWDGE queue: drains in parallel with the sync-queue loads
                nc.gpsimd.dma_start(
                    out=out[:, t * P:(t + 1) * P, :].rearrange("b p j -> p b j"),
                    in_=outt[:].rearrange("p (b j) -> p b j", j=B),
                )
    if not nc.is_finalized():
        nc.finalize()
    return nc


_NC = None
last_results = None


def _get_nc():
    global _NC
    if _NC is None:
        _NC = _build_nc()
    return _NC


def kernel(support_set: np.ndarray, input_signal: np.ndarray) -> np.ndarray:
    global last_results
    support_set = np.ascontiguousarray(support_set, dtype=np.float32)
    input_signal = np.ascontiguousarray(input_signal, dtype=np.float32)
    nc = _get_nc()
    in_maps = [
        {
            "support": np.ascontiguousarray(support_set[i * SL:(i + 1) * SL]),
            "inp": input_signal,
        }
        for i in range(NCORES)
    ]
    res = run_bass_kernel_spmd(nc, in_maps, list(range(NCORES)))
    last_results = res
    return np.concatenate([res.results[i]["out"] for i in range(NCORES)], axis=1)


# revision 22
# speedup vs baseline: 1.0465x; 1.0465x over previous
"""Trainium2 Bass kernel for nn_DistanceNetwork (retrieval_knn).

out[b, s, j] = dot[s, j] / (||sup[s, b]|| * ||inp[b]|| + EPS)
  dot[s, j] = sum_d sup[s, j, d] * inp[j, d]

Sharding: S=8192 split across 8 cores (1024 each). Each core reads its
support slice + the full input_signal, writes its [B, 1024, B] output
slice; host concatenates along axis 1.

Engine split per 128-s tile (layout [128 part = s, free = (b d)]):
 - DVE: fused mul+cumsum custom op (DOT_SCAN) -> per-segment dot via
   strided cumsum differences; SQ_SCAN cumsum of squares for the first
   K_DVE b-segments; small fixup ops.
 - ACT: Square+accumulate for the remaining b-segments' norms; sqrt.
 - GpSimd: the [B,B] outer-product broadcast multiply.
 - HWDGE (sync) DMAs.
"""

import os
import sys

import numpy as np

for _p in ("/opt/trn_rl_repo", "/root/.axon_site/_ro/trn_rl_repo"):
    if os.path.isdir(_p) and _p not in sys.path:
        sys.path.insert(0, _p)

import concourse.bass as bass
import concourse.bacc as bacc
import concourse.mybir as mybir
from concourse.bass_utils import run_bass_kernel_spmd, dve_ver_for
from concourse.tile import TileContext

S, B, D = 8192, 32, 128
NCORES = 8
SL = S // NCORES          # 1024 s-rows per core
P = 128                   # partition tile of s
TILES = SL // P           # 8 s-tiles per core
BD = B * D                # 4096
EPS = 1e-10
F32 = mybir.dt.float32
X = mybir.AxisListType.X

# How many of the 32 b-segments' sum-of-squares DVE computes (via SQ_SCAN);
# the rest go to the Scalar engine as Square+accumulate chunks.
K_DVE = 19
KD = K_DVE * D


# --- custom DVE ops (registered at import; uop table is built per-NEFF) --- #

def _register_scan_ops():
    import concourse.dve_ops as dve_ops_mod
    from concourse.dve_ops import DveOp, OPS, CUSTOM_DVE_SPECS
    from concourse.dve_spec import Spec, Src0, Src1, AluOp, scan, sq, lower
    from concourse.dve_spec import _has_src1
    from concourse.dve_uop import DveOpSpec

    def reg(name, spec):
        if name in dve_ops_mod._SUB_OPCODE_FOR_NAME:
            return next(op for op in OPS if op.name == name)
        op = DveOp(name=name, spec=spec, subdim=False, uops_sha={})
        OPS.append(op)
        CUSTOM_DVE_SPECS[name] = spec
        row = dve_ops_mod._CUSTOM_DVE_ROW_BASE + len(OPS) - 1
        assert row < 0x20
        dve_ops_mod._SUB_OPCODE_FOR_NAME[name] = row
        for ver in ("v3", "v4"):
            try:
                spec_c = DveOpSpec(
                    name=name,
                    opcode=row,
                    uops=lower(spec, ver=ver),
                    rd1_en=_has_src1(spec),
                )
                op.uops_sha[ver] = spec_c.sha(ver)
            except Exception:
                pass
        return op

    dot_scan = reg(
        "ANTK_DOT_SCAN",
        Spec(
            body=scan(AluOp.ADD, Src0 * Src1),
            reference=lambda in0, in1, s0, s1, imm2: np.cumsum(
                in0.astype(np.float32) * in1.astype(np.float32), axis=-1
            ),
        ),
    )
    sq_scan = reg(
        "ANTK_SQ_SCAN",
        Spec(
            body=scan(AluOp.ADD, sq(Src0)),
            reference=lambda in0, in1, s0, s1, imm2: np.cumsum(
                np.square(in0.astype(np.float32)), axis=-1
            ),
        ),
    )
    return dot_scan, sq_scan


DOT_SCAN, SQ_SCAN = _register_scan_ops()


def _build_nc():
    nc = bacc.Bacc()
    sup = nc.declare_dram_parameter("support", [SL, B, D], F32, isOutput=False)
    inp = nc.declare_dram_parameter("inp", [B, D], F32, isOutput=False)
    out = nc.declare_dram_parameter("out", [B, SL, B], F32, isOutput=True)
    SQUARE = mybir.ActivationFunctionType.Square

    with TileContext(nc) as tc:
        with (
            tc.tile_pool(name="const", bufs=1) as cpool,
            tc.tile_pool(name="sup", bufs=TILES) as suppool,
            tc.tile_pool(name="scan", bufs=2) as scpool,
            tc.tile_pool(name="outp", bufs=2) as opool,
            tc.tile_pool(name="small", bufs=3) as spool,
            tc.tile_pool(name="psum", bufs=1, space="PSUM") as ppool,
        ):
            # input_signal broadcast to all 128 partitions: [128, (b d)].
            # Read the 16 KiB once from HBM, then replicate across partitions
            # with K=1 ones-matmuls into PSUM (PE is otherwise idle; saves
            # both HBM broadcast traffic and 16 KiB/partition of SBUF).
            ones_l = cpool.tile([1, P], F32)
            nc.gpsimd.memset(ones_l[:], 1.0)
            inp_rep = ppool.tile([P, BD], F32)
            NBANK = 512
            # dummy matmul: eats the PE cold-start before inp_one arrives
            nc.tensor.matmul(
                inp_rep[0:1, 0:1], ones_l[:, 0:1], ones_l[:, 0:1],
                start=True, stop=True,
            )
            inp_one = scpool.tile([1, BD], F32, tag="dscan")
            with tc.high_priority():
                nc.sync.dma_start(
                    out=inp_one[:],
                    in_=inp[:, :].rearrange("b d -> (b d)").unsqueeze(0),
                )
                for k in range(BD // NBANK):
                    nc.tensor.matmul(
                        inp_rep[:, k * NBANK:(k + 1) * NBANK],
                        ones_l[:],
                        inp_one[:, k * NBANK:(k + 1) * NBANK],
                        start=True,
                        stop=True,
                    )
            tnorm = cpool.tile([P, B], F32)

            for t in range(TILES):
                sup_t = suppool.tile([P, BD], F32, tag="sup")
                nc.sync.dma_start(
                    out=sup_t[:],
                    in_=sup[t * P:(t + 1) * P, :, :].rearrange("s b d -> s (b d)"),
                )

                # dot[p, j]: cumsum of sup*inp along (b d); per-segment sums
                # are differences of the padded cumsum at segment boundaries.
                dsc = scpool.tile([P, BD + 1], F32, tag="dscan")
                nc.gpsimd.memset(dsc[:, 0:1], 0.0)
                nc.vector._custom_dve(
                    DOT_SCAN, out=dsc[:, 1:BD + 1], in0=sup_t[:], in1=inp_rep[:]
                )
                dot = spool.tile([P, B], F32, tag="dot")
                ends = dsc[:, 1:BD + 1].rearrange("p (b d) -> p b d", d=D)
                prevs = dsc[:, 0:BD].rearrange("p (b d) -> p b d", d=D)
                nc.gpsimd.tensor_sub(
                    dot[:], ends[:, :, D - 1:D].squeeze(2), prevs[:, :, 0:1].squeeze(2)
                )

                # sq[p, b]: first K_DVE segments on DVE (cumsum of squares),
                # the rest on ACT (Square with accumulate), 128 elems each.
                sq = spool.tile([P, B], F32, tag="sq")
                ssc = scpool.tile([P, KD + 1], F32, tag="sscan")
                nc.gpsimd.memset(ssc[:, 0:1], 0.0)
                nc.vector._custom_dve(
                    SQ_SCAN, out=ssc[:, 1:KD + 1], in0=sup_t[:, 0:KD]
                )
                sends = ssc[:, 1:KD + 1].rearrange("p (b d) -> p b d", d=D)
                sprevs = ssc[:, 0:KD].rearrange("p (b d) -> p b d", d=D)
                nc.gpsimd.tensor_sub(
                    sq[:, 0:K_DVE],
                    sends[:, :, D - 1:D].squeeze(2),
                    sprevs[:, :, 0:1].squeeze(2),
                )
                scr = spool.tile([P, D], F32, tag="scr")
                for b in range(K_DVE, B):
                    nc.scalar.activation(
                        scr[:],
                        sup_t[:, b * D:(b + 1) * D],
                        SQUARE,
                        accum_out=sq[:, b:b + 1],
                    )


                if t == 0:
                    # tnorm[p, b] = ||inp[b]|| — emitted after tile-0's main
                    # ops so the scheduler doesn't stall tile 0 behind it
                    itmp = scpool.tile([P, BD + 1], F32, tag="dscan")
                    nc.gpsimd.memset(itmp[:, 0:1], 0.0)
                    nc.vector._custom_dve(
                        SQ_SCAN, out=itmp[:, 1:BD + 1], in0=inp_rep[:]
                    )
                    tn2 = cpool.tile([P, B], F32)
                    iends = itmp[:, 1:BD + 1].rearrange("p (b d) -> p b d", d=D)
                    iprevs = itmp[:, 0:BD].rearrange("p (b d) -> p b d", d=D)
                    nc.gpsimd.tensor_sub(
                        tn2[:],
                        iends[:, :, D - 1:D].squeeze(2),
                        iprevs[:, :, 0:1].squeeze(2),
                    )
                    nc.scalar.sqrt(tnorm[:], tn2[:])

                # rden = 1 / ((sqrt(sq) + EPS') * tnorm)  (EPS folded in)
                sn = spool.tile([P, B], F32, tag="sn")
                nc.scalar.sqrt(sn[:], sq[:])
                den = spool.tile([P, B], F32, tag="den")
                nc.vector.scalar_tensor_tensor(
                    out=den[:],
                    in0=sn[:],
                    scalar=EPS,
                    in1=tnorm[:],
                    op0=mybir.AluOpType.add,
                    op1=mybir.AluOpType.mult,
                )
                rden = spool.tile([P, B], F32, tag="rden")
                nc.vector.reciprocal(rden[:], den[:])

                # outt[p, b, j] = rden[p, b] * dot[p, j]   (GpSimd)
                outt = opool.tile([P, B * B], F32, tag="outt")
                nc.gpsimd.tensor_mul(
                    outt[:].rearrange("p (b j) -> p b j", j=B),
                    rden[:].unsqueeze(2).broadcast_to([P, B, B]),
                    dot[:].unsqueeze(1).broadcast_to([P, B, B]),
                )
                # SWDGE queue: drains in parallel with the sync-queue loads
                nc.gpsimd.dma_start(
                    out=out[:, t * P:(t + 1) * P, :].rearrange("b p j -> p b j"),
                    in_=outt[:].rearrange("p (b j) -> p b j", j=B),
                )
    if not nc.is_finalized():
        nc.finalize()
    return nc


_NC = None
last_results = None


def _get_nc():
    global _NC
    if _NC is None:
        _NC = _build_nc()
    return _NC


def kernel(support_set: np.ndarray, input_signal: np.ndarray) -> np.ndarray:
    global last_results
    support_set = np.ascontiguousarray(support_set, dtype=np.float32)
    input_signal = np.ascontiguousarray(input_signal, dtype=np.float32)
    nc = _get_nc()
    in_maps = [
        {
            "support": np.ascontiguousarray(support_set[i * SL:(i + 1) * SL]),
            "inp": input_signal,
        }
        for i in range(NCORES)
    ]
    res = run_bass_kernel_spmd(nc, in_maps, list(range(NCORES)))
    last_results = res
    return np.concatenate([res.results[i]["out"] for i in range(NCORES)], axis=1)


# revision 23
# speedup vs baseline: 1.0986x; 1.0498x over previous
"""Trainium2 Bass kernel for nn_DistanceNetwork (retrieval_knn).

out[b, s, j] = dot[s, j] / (||sup[s, b]|| * ||inp[b]|| + EPS)
  dot[s, j] = sum_d sup[s, j, d] * inp[j, d]

Sharding: S=8192 split across 8 cores (1024 each). Each core reads its
support slice + the full input_signal, writes its [B, 1024, B] output
slice; host concatenates along axis 1.

Engine split per 128-s tile (layout [128 part = s, free = (b d)]):
 - DVE: fused mul+cumsum custom op (DOT_SCAN) -> per-segment dot via
   strided cumsum differences; SQ_SCAN cumsum of squares for the first
   K_DVE b-segments; small fixup ops.
 - ACT: Square+accumulate for the remaining b-segments' norms; sqrt.
 - GpSimd: the [B,B] outer-product broadcast multiply.
 - HWDGE (sync) DMAs.
"""

import os
import sys

import numpy as np

for _p in ("/opt/trn_rl_repo", "/root/.axon_site/_ro/trn_rl_repo"):
    if os.path.isdir(_p) and _p not in sys.path:
        sys.path.insert(0, _p)

import concourse.bass as bass
import concourse.bacc as bacc
import concourse.mybir as mybir
from concourse.bass_utils import run_bass_kernel_spmd, dve_ver_for
from concourse.tile import TileContext

S, B, D = 8192, 32, 128
NCORES = 8
SL = S // NCORES          # 1024 s-rows per core
P = 128                   # partition tile of s
TILES = SL // P           # 8 s-tiles per core
BD = B * D                # 4096
EPS = 1e-10
F32 = mybir.dt.float32
X = mybir.AxisListType.X

# How many of the 32 b-segments' sum-of-squares DVE computes (via SQ_SCAN);
# the rest go to the Scalar engine as Square+accumulate chunks.
K_DVE = 19
KD = K_DVE * D


# --- custom DVE ops (registered at import; uop table is built per-NEFF) --- #

def _register_scan_ops():
    import concourse.dve_ops as dve_ops_mod
    from concourse.dve_ops import DveOp, OPS, CUSTOM_DVE_SPECS
    from concourse.dve_spec import Spec, Src0, Src1, AluOp, scan, sq, lower
    from concourse.dve_spec import _has_src1
    from concourse.dve_uop import DveOpSpec

    def reg(name, spec):
        if name in dve_ops_mod._SUB_OPCODE_FOR_NAME:
            return next(op for op in OPS if op.name == name)
        op = DveOp(name=name, spec=spec, subdim=False, uops_sha={})
        OPS.append(op)
        CUSTOM_DVE_SPECS[name] = spec
        row = dve_ops_mod._CUSTOM_DVE_ROW_BASE + len(OPS) - 1
        assert row < 0x20
        dve_ops_mod._SUB_OPCODE_FOR_NAME[name] = row
        for ver in ("v3", "v4"):
            try:
                spec_c = DveOpSpec(
                    name=name,
                    opcode=row,
                    uops=lower(spec, ver=ver),
                    rd1_en=_has_src1(spec),
                )
                op.uops_sha[ver] = spec_c.sha(ver)
            except Exception:
                pass
        return op

    dot_scan = reg(
        "ANTK_DOT_SCAN",
        Spec(
            body=scan(AluOp.ADD, Src0 * Src1),
            reference=lambda in0, in1, s0, s1, imm2: np.cumsum(
                in0.astype(np.float32) * in1.astype(np.float32), axis=-1
            ),
        ),
    )
    sq_scan = reg(
        "ANTK_SQ_SCAN",
        Spec(
            body=scan(AluOp.ADD, sq(Src0)),
            reference=lambda in0, in1, s0, s1, imm2: np.cumsum(
                np.square(in0.astype(np.float32)), axis=-1
            ),
        ),
    )
    return dot_scan, sq_scan


DOT_SCAN, SQ_SCAN = _register_scan_ops()


def _build_nc():
    nc = bacc.Bacc()
    sup = nc.declare_dram_parameter("support", [SL, B, D], F32, isOutput=False)
    inp = nc.declare_dram_parameter("inp", [B, D], F32, isOutput=False)
    tnh = nc.declare_dram_parameter("tnorm", [1, B], F32, isOutput=False)
    out = nc.declare_dram_parameter("out", [B, SL, B], F32, isOutput=True)
    SQUARE = mybir.ActivationFunctionType.Square

    with TileContext(nc) as tc:
        with (
            tc.tile_pool(name="const", bufs=1) as cpool,
            tc.tile_pool(name="sup", bufs=TILES) as suppool,
            tc.tile_pool(name="scan", bufs=2) as scpool,
            tc.tile_pool(name="outp", bufs=2) as opool,
            tc.tile_pool(name="small", bufs=3) as spool,
            tc.tile_pool(name="psum", bufs=1, space="PSUM") as ppool,
        ):
            # input_signal broadcast to all 128 partitions: [128, (b d)].
            # Read the 16 KiB once from HBM, then replicate across partitions
            # with K=1 ones-matmuls into PSUM (PE is otherwise idle; saves
            # both HBM broadcast traffic and 16 KiB/partition of SBUF).
            ones_l = cpool.tile([1, P], F32)
            nc.gpsimd.memset(ones_l[:], 1.0)
            inp_rep = ppool.tile([P, BD], F32)
            NBANK = 512
            # dummy matmul: eats the PE cold-start before inp_one arrives
            nc.tensor.matmul(
                inp_rep[0:1, 0:1], ones_l[:, 0:1], ones_l[:, 0:1],
                start=True, stop=True,
            )
            inp_one = scpool.tile([1, BD], F32, tag="dscan")
            tnorm = cpool.tile([P, B], F32)
            with tc.high_priority():
                nc.scalar.dma_start(
                    out=inp_one[:],
                    in_=inp[:, :].rearrange("b d -> (b d)").unsqueeze(0),
                )
                nc.scalar.dma_start(
                    out=tnorm[:], in_=tnh[:, :].broadcast_to([P, B])
                )
                for k in range(BD // NBANK):
                    nc.tensor.matmul(
                        inp_rep[:, k * NBANK:(k + 1) * NBANK],
                        ones_l[:],
                        inp_one[:, k * NBANK:(k + 1) * NBANK],
                        start=True,
                        stop=True,
                    )
            for t in range(TILES):
                sup_t = suppool.tile([P, BD], F32, tag="sup")
                nc.sync.dma_start(
                    out=sup_t[:],
                    in_=sup[t * P:(t + 1) * P, :, :].rearrange("s b d -> s (b d)"),
                )

                # dot[p, j]: cumsum of sup*inp along (b d); per-segment sums
                # are differences of the padded cumsum at segment boundaries.
                dsc = scpool.tile([P, BD + 1], F32, tag="dscan")
                nc.gpsimd.memset(dsc[:, 0:1], 0.0)
                nc.vector._custom_dve(
                    DOT_SCAN, out=dsc[:, 1:BD + 1], in0=sup_t[:], in1=inp_rep[:]
                )
                dot = spool.tile([P, B], F32, tag="dot")
                ends = dsc[:, 1:BD + 1].rearrange("p (b d) -> p b d", d=D)
                prevs = dsc[:, 0:BD].rearrange("p (b d) -> p b d", d=D)
                nc.gpsimd.tensor_sub(
                    dot[:], ends[:, :, D - 1:D].squeeze(2), prevs[:, :, 0:1].squeeze(2)
                )

                # sq[p, b]: first K_DVE segments on DVE (cumsum of squares),
                # the rest on ACT (Square with accumulate), 128 elems each.
                sq = spool.tile([P, B], F32, tag="sq")
                ssc = scpool.tile([P, KD + 1], F32, tag="sscan")
                nc.gpsimd.memset(ssc[:, 0:1], 0.0)
                nc.vector._custom_dve(
                    SQ_SCAN, out=ssc[:, 1:KD + 1], in0=sup_t[:, 0:KD]
                )
                sends = ssc[:, 1:KD + 1].rearrange("p (b d) -> p b d", d=D)
                sprevs = ssc[:, 0:KD].rearrange("p (b d) -> p b d", d=D)
                nc.gpsimd.tensor_sub(
                    sq[:, 0:K_DVE],
                    sends[:, :, D - 1:D].squeeze(2),
                    sprevs[:, :, 0:1].squeeze(2),
                )
                scr = spool.tile([P, D], F32, tag="scr")
                for b in range(K_DVE, B):
                    nc.scalar.activation(
                        scr[:],
                        sup_t[:, b * D:(b + 1) * D],
                        SQUARE,
                        accum_out=sq[:, b:b + 1],
                    )


                # rden = 1 / ((sqrt(sq) + EPS') * tnorm)  (EPS folded in)
                sn = spool.tile([P, B], F32, tag="sn")
                nc.scalar.sqrt(sn[:], sq[:])
                den = spool.tile([P, B], F32, tag="den")
                nc.vector.scalar_tensor_tensor(
                    out=den[:],
                    in0=sn[:],
                    scalar=EPS,
                    in1=tnorm[:],
                    op0=mybir.AluOpType.add,
                    op1=mybir.AluOpType.mult,
                )
                rden = spool.tile([P, B], F32, tag="rden")
                nc.vector.reciprocal(rden[:], den[:])

                # outt[p, b, j] = rden[p, b] * dot[p, j]   (GpSimd)
                outt = opool.tile([P, B * B], F32, tag="outt")
                nc.gpsimd.tensor_mul(
                    outt[:].rearrange("p (b j) -> p b j", j=B),
                    rden[:].unsqueeze(2).broadcast_to([P, B, B]),
                    dot[:].unsqueeze(1).broadcast_to([P, B, B]),
                )
                # SWDGE queue: drains in parallel with the sync-queue loads
                nc.gpsimd.dma_start(
                    out=out[:, t * P:(t + 1) * P, :].rearrange("b p j -> p b j"),
                    in_=outt[:].rearrange("p (b j) -> p b j", j=B),
                )
    if not nc.is_finalized():
        nc.finalize()
    return nc


_NC = None
last_results = None


def _get_nc():
    global _NC
    if _NC is None:
        _NC = _build_nc()
    return _NC


def kernel(support_set: np.ndarray, input_signal: np.ndarray) -> np.ndarray:
    global last_results
    support_set = np.ascontiguousarray(support_set, dtype=np.float32)
    input_signal = np.ascontiguousarray(input_signal, dtype=np.float32)
    nc = _get_nc()
    tnorm = np.sqrt(np.sum(input_signal.astype(np.float32) ** 2, axis=1))
    tnorm = np.ascontiguousarray(tnorm.reshape(1, B), dtype=np.float32)
    in_maps = [
        {
            "support": np.ascontiguousarray(support_set[i * SL:(i + 1) * SL]),
            "inp": input_signal,
            "tnorm": tnorm,
        }
        for i in range(NCORES)
    ]
    res = run_bass_kernel_spmd(nc, in_maps, list(range(NCORES)))
    last_results = res
    return np.concatenate([res.results[i]["out"] for i in range(NCORES)], axis=1)
